# revision 1
# baseline (speedup 1.0000x reference)
"""Multi-head self-attention Trainium2 Bass kernel (B=2, T=4096, D=512, H=8).

Sharding: 8 cores, each handles (batch b = core//4, head-pair hp = core%4).
Per core, for its 2 heads (host pre-transposes x and pre-scales Wq by 1/8):
    qT = Wq' @ x.T + bq'    kT = Wk @ x.T + bk     ([128, T]: head h on
                                                    partitions 64h..64h+63)
    v  = x @ Wv.T                                  ([T, 2*64], interleaved
                                                    with ones columns)
    flash attention without max-subtraction (scores ~N(0,1), f32 exp safe):
        S.T chunk = k_kb @ qT                ([128 kv, QS q] PSUM)
        P.T = exp(S.T)                       (one ACT op per chunk)
        ctxT[+l] += vaug_kb.T @ P.T          ([66, 512] PSUM accumulators,
                                              rows 0..63 ctx.T, 64..65 = l)
    normalize: 1/l (DVE) -> DRAM round-trip stride-0 DMA broadcast ->
        DVE multiply (no PE involvement)
    partial_out = ctx2 @ Wo[:, hp].T         ([T, 512] f32)
Host gathers: out[b] = sum of 4 cores' partials + (bv @ Wo.T + bo); the
v/o biases fold out exactly because softmax rows sum to 1.

All matmul operands are float32r (TF32-ish, ~1e-4 rel err, 1 cycle/row on
the PE at N>=256 vs 4 for fp32). This walrus build accepts at most ONE sync
wait per instruction; split_excess_waits() moves extras onto no-ops.
walrus's LDWEIGHTS-dedup pass is re-enabled (run_command patch) and matmuls
sharing a stationary operand are emitted adjacently so the reload elides.
"""

import numpy as np

import concourse.bass as bass
import concourse.tile as tile
from concourse import mybir
from concourse.bass_utils import run_bass_kernel_spmd
from concourse import bass_utils as _bu

if not getattr(_bu, "_ldw_opt_patch", False):
    _orig_run_command = _bu.run_command

    def _patched_run_command(argv, **kw):
        argv = ["--enable-ldw-opt=true" if a == "--enable-ldw-opt=false" else a
                for a in argv]
        return _orig_run_command(argv, **kw)

    _bu.run_command = _patched_run_command
    _bu._ldw_opt_patch = True

F32R = mybir.dt.float32r
F32 = mybir.dt.float32

N_CORES = 8
B, T, D, H = 2, 4096, 512, 8
DK = D // H          # 64
TT = T // 128        # 32 kv tiles
KC = D // 128        # 4 contraction chunks
QS = 1024            # q super-block (exp granularity)
NC2 = QS // 512      # 512-wide q chunks per super
NQS = T // QS        # supers per head
VW = 132             # vaug cols per kv tile: [v_h0(64) one one v_h1(64) one one]

_split_ctr = [0]


def split_excess_waits(nc, limit=1):
    """walrus codegen in this toolchain accepts at most `limit` sync waits
    per instruction; move the excess onto nofuse NoOps inserted right before
    on the same engine (engines execute in order, semantics unchanged)."""
    n_split = 0
    for fn in nc.m.functions:
        blocks = fn.blocks if isinstance(fn.blocks, list) else list(fn.blocks.values())
        for blk in blocks:
            out = []
            for inst in blk.instructions:
                si = inst.sync_info
                if si is not None and len(si.on_wait) > limit:
                    waits = list(si.on_wait)
                    excess, keep = waits[:-limit], waits[-limit:]
                    for w in excess:
                        _split_ctr[0] += 1
                        out.append(mybir.InstNoOp(
                            name=f"I-wsplit-{_split_ctr[0]}",
                            opcode="NoOp",
                            engine=inst.engine,
                            sync_info=mybir.SyncInfo(on_wait=[w], on_update=[]),
                            bass_nofuse=True,
                        ))
                        n_split += 1
                    inst.sync_info = mybir.SyncInfo(
                        on_wait=keep, on_update=list(si.on_update))
                out.append(inst)
            blk.instructions[:] = out
    return n_split


def _bcast_ap(src_row, nparts):
    """Stride-0 partition broadcast view of a [1, N] AP (DRAM source only)."""
    return bass.AP(
        tensor=src_row.tensor,
        offset=src_row.offset,
        ap=[[0, nparts]] + [list(d) for d in src_row.ap[1:]],
    )


def build_kernel():
    nc = bass.Bass()
    xbT = nc.dram_tensor("xbT", [D, T], F32R, kind="ExternalInput")
    wqT = nc.dram_tensor("wqT", [D, 128], F32R, kind="ExternalInput")
    wkT = nc.dram_tensor("wkT", [D, 128], F32R, kind="ExternalInput")
    wvT = nc.dram_tensor("wvT", [D, 128], F32R, kind="ExternalInput")
    woT = nc.dram_tensor("woT", [128, D], F32R, kind="ExternalInput")
    bq = nc.dram_tensor("bq", [128, 1], F32, kind="ExternalInput")
    bk = nc.dram_tensor("bk", [128, 1], F32, kind="ExternalInput")
    part = nc.dram_tensor("part", [T, D], F32, kind="ExternalOutput")

    with tile.TileContext(nc) as tc:
        with tc.tile_pool(name="persist", bufs=1) as persist:
            # ---- persistent SBUF ----
            woTs = persist.tile([128, D], F32R)
            nc.sync.dma_start(out=woTs, in_=woT[:, :])
            bq_t = persist.tile([128, 1], F32)
            nc.sync.dma_start(out=bq_t, in_=bq[:, :])
            bk_t = persist.tile([128, 1], F32)
            nc.sync.dma_start(out=bk_t, in_=bk[:, :])
            wqt = persist.tile([128, KC, 128], F32R)
            nc.sync.dma_start(out=wqt, in_=wqT.rearrange("(c p) m -> p c m", p=128))
            wkt = persist.tile([128, KC, 128], F32R)
            nc.sync.dma_start(out=wkt, in_=wkT.rearrange("(c p) m -> p c m", p=128))
            wvt = persist.tile([128, KC, 128], F32R)
            nc.sync.dma_start(out=wvt, in_=wvT.rearrange("(c p) m -> p c m", p=128))
            ones2 = persist.tile([128, 2], F32)
            nc.vector.memset(ones2, 1.0)

            qT2 = persist.tile([128, T], F32R)   # heads stacked [h0|h1]
            kT2 = persist.tile([128, T], F32R)
            vaug = persist.tile([128, TT * VW], F32R)
            ctxT2 = persist.tile([128, T], F32R)

            # ---- stage A/B: load xT (chunked, pipelined) + projections ----
            with tc.tile_pool(name="xT", bufs=1) as xTp:
                xTall = xTp.tile([128, KC * T], F32R)  # chunk c at cols [c*T,...)
                with tc.tile_pool(name="psB", bufs=2, space="PSUM") as psB, \
                     tc.tile_pool(name="psV", bufs=2, space="PSUM") as psV:
                    for n in range(T // 512):
                        sl = slice(512 * n, 512 * (n + 1))
                        for c in range(KC):
                            nc.sync.dma_start(
                                out=xTall[:, c * T + 512 * n: c * T + 512 * (n + 1)],
                                in_=xbT[128 * c: 128 * (c + 1), sl])
                        ps_q = psB.tile([128, 512], F32, tag="psq")
                        for c in range(KC):
                            nc.tensor.matmul(
                                ps_q, wqt[:, c, :],
                                xTall[:, c * T + 512 * n: c * T + 512 * (n + 1)],
                                start=(c == 0), stop=(c == KC - 1))
                        nc.vector.tensor_scalar_add(
                            out=qT2[:, sl], in0=ps_q, scalar1=bq_t)
                        ps_k = psB.tile([128, 512], F32, tag="psk")
                        for c in range(KC):
                            nc.tensor.matmul(
                                ps_k, wkt[:, c, :],
                                xTall[:, c * T + 512 * n: c * T + 512 * (n + 1)],
                                start=(c == 0), stop=(c == KC - 1))
                        nc.vector.tensor_scalar_add(
                            out=kT2[:, sl], in0=ps_k, scalar1=bk_t)
                    for i in range(TT):
                        ps_v = psV.tile([128, 128], F32, tag="psv")
                        for c in range(KC):
                            nc.tensor.matmul(
                                ps_v,
                                xTall[:, c * T + 128 * i: c * T + 128 * (i + 1)],
                                wvt[:, c, :],
                                start=(c == 0), stop=(c == KC - 1))
                        nc.vector.tensor_copy(
                            out=vaug[:, VW * i: VW * i + 64], in_=ps_v[:, 0:64])
                        nc.vector.tensor_copy(
                            out=vaug[:, VW * i + 66: VW * i + 130],
                            in_=ps_v[:, 64:128])
                        nc.vector.tensor_copy(
                            out=vaug[:, VW * i + 64: VW * i + 66], in_=ones2)
                        nc.vector.tensor_copy(
                            out=vaug[:, VW * i + 130: VW * i + 132], in_=ones2)

            # ---- stage C: flash attention per head ----
            with tc.tile_pool(name="stp", bufs=2, space="PSUM") as stp, \
                 tc.tile_pool(name="ctxp", bufs=2, space="PSUM") as ctxp, \
                 tc.tile_pool(name="ptp", bufs=8) as ptp, \
                 tc.tile_pool(name="drp", bufs=4, space="DRAM") as drp, \
                 tc.tile_pool(name="sC", bufs=4) as sC:
                for h in range(2):
                    hq = 64 * h
                    for qi in range(NQS):
                        qoff = QS * qi
                        ps_cts = []
                        for c2 in range(NC2):
                            ps_cts.append(ctxp.tile(
                                [66, 512], F32, tag=f"ctxT{c2}",
                                name=f"psct_{h}_{qi}_{c2}"))
                        for kb in range(TT):
                            st = stp.tile([128, QS], F32, tag="st")
                            for c2 in range(NC2):
                                nc.tensor.matmul(
                                    st[:, 512 * c2: 512 * (c2 + 1)],
                                    kT2[hq:hq + 64, 128 * kb: 128 * (kb + 1)],
                                    qT2[hq:hq + 64,
                                        qoff + 512 * c2: qoff + 512 * (c2 + 1)],
                                    start=True, stop=True)
                            pt = ptp.tile([128, QS], F32R, tag="pt")
                            nc.scalar.activation(
                                out=pt, in_=st,
                                func=mybir.ActivationFunctionType.Exp)
                            for c2 in range(NC2):
                                nc.tensor.matmul(
                                    ps_cts[c2],
                                    vaug[:, VW * kb + 66 * h: VW * kb + 66 * h + 66],
                                    pt[:, 512 * c2: 512 * (c2 + 1)],
                                    start=(kb == 0), stop=(kb == TT - 1))
                        for c2 in range(NC2):
                            rec2 = sC.tile([2, 512], F32, tag="rec2")
                            nc.vector.reciprocal(rec2, ps_cts[c2][64:66, :])
                            drec = drp.tile([2, 512], F32, tag="drec")
                            nc.sync.dma_start(out=drec, in_=rec2)
                            rbc = sC.tile([64, 512], F32, tag="rbc")
                            nc.gpsimd.dma_start(
                                out=rbc, in_=_bcast_ap(drec[0:1, :], 64))
                            nc.vector.tensor_mul(
                                out=ctxT2[hq:hq + 64,
                                          qoff + 512 * c2: qoff + 512 * (c2 + 1)],
                                in0=ps_cts[c2][0:64, :], in1=rbc)

            # ---- stage D: output projection ----
            with tc.tile_pool(name="psD", bufs=3, space="PSUM") as psD, \
                 tc.tile_pool(name="sD", bufs=4) as sD:
                for g in range(8):
                    ost = sD.tile([128, 4, 512], F32, tag="ost")
                    for a in range(4):
                        i = 4 * g + a
                        ps_o = psD.tile([128, 512], F32, tag="pso")
                        nc.tensor.matmul(
                            ps_o, ctxT2[:, 128 * i: 128 * (i + 1)], woTs,
                            start=True, stop=True)
                        nc.vector.tensor_copy(out=ost[:, a, :], in_=ps_o)
                    nc.sync.dma_start(
                        out=part[512 * g: 512 * (g + 1), :].rearrange(
                            "(a p) d -> p a d", p=128),
                        in_=ost)

    split_excess_waits(nc)
    return nc


_NC_CACHE = None


def _get_nc():
    global _NC_CACHE
    if _NC_CACHE is None:
        _NC_CACHE = build_kernel()
    return _NC_CACHE


def make_in_maps(x, Wq, bq, Wk, bk, Wv, bv, Wo, bo):
    scale = 1.0 / np.sqrt(DK)
    in_maps = []
    for core in range(N_CORES):
        b, hp = divmod(core, 4)
        R = slice(128 * hp, 128 * hp + 128)
        in_maps.append({
            "xbT": np.ascontiguousarray(x[b].T, dtype=np.float32),
            "wqT": np.ascontiguousarray((Wq[R] * scale).T, dtype=np.float32),
            "wkT": np.ascontiguousarray(Wk[R].T, dtype=np.float32),
            "wvT": np.ascontiguousarray(Wv[R].T, dtype=np.float32),
            "woT": np.ascontiguousarray(Wo[:, R].T, dtype=np.float32),
            "bq": np.ascontiguousarray(
                (bq[R] * scale).reshape(128, 1), dtype=np.float32),
            "bk": np.ascontiguousarray(bk[R].reshape(128, 1), dtype=np.float32),
        })
    return in_maps


def kernel(x, Wq, bq, Wk, bk, Wv, bv, Wo, bo):
    x = np.asarray(x, dtype=np.float32)
    Wq, Wk, Wv, Wo = (np.asarray(a, dtype=np.float32) for a in (Wq, Wk, Wv, Wo))
    bq, bk, bv, bo = (np.asarray(a, dtype=np.float32) for a in (bq, bk, bv, bo))

    nc = _get_nc()
    in_maps = make_in_maps(x, Wq, bq, Wk, bk, Wv, bv, Wo, bo)
    res = run_bass_kernel_spmd(nc, in_maps, list(range(N_CORES)))
    parts = [res.results[c]["part"] for c in range(N_CORES)]

    bcorr = (bv @ Wo.T + bo).astype(np.float32)  # exact bv/bo contribution
    out = np.empty((B, T, D), dtype=np.float32)
    for b in range(B):
        acc = parts[4 * b].astype(np.float64)
        for c in range(4 * b + 1, 4 * b + 4):
            acc += parts[c]
        out[b] = (acc + bcorr).astype(np.float32)
    return out



# revision 5
# speedup vs baseline: 1.6243x; 1.6243x over previous
"""Multi-head self-attention Trainium2 Bass kernel (B=2, T=4096, D=512, H=8).

Sharding: 8 cores, each handles (batch b = core//4, head-pair hp = core%4).
Per core, for its 2 heads (host pre-transposes x and pre-scales Wq by 1/8):
    qT = Wq' @ x.T + bq'    kT = Wk @ x.T + bk     ([128, T]: head h on
                                                    partitions 64h..64h+63)
    v  = x @ Wv.T                                  ([T, 2*64], interleaved
                                                    with ones columns)
    flash attention without max-subtraction (scores ~N(0,1), f32 exp safe):
        S.T chunk = k_kb @ qT                ([128 kv, QS q] PSUM)
        P.T = exp(S.T)                       (one ACT op per chunk)
        ctxT[+l] += vaug_kb.T @ P.T          ([66, 512] PSUM accumulators,
                                              rows 0..63 ctx.T, 64..65 = l)
    normalize: 1/l (DVE) -> DRAM round-trip stride-0 DMA broadcast ->
        DVE multiply (no PE involvement)
    partial_out = ctx2 @ Wo[:, hp].T         ([T, 512] f32)
Host gathers: out[b] = sum of 4 cores' partials + (bv @ Wo.T + bo); the
v/o biases fold out exactly because softmax rows sum to 1.

All matmul operands are float32r (TF32-ish, ~1e-4 rel err, 1 cycle/row on
the PE at N>=256 vs 4 for fp32). This walrus build accepts at most ONE sync
wait per instruction; split_excess_waits() moves extras onto no-ops.
walrus's LDWEIGHTS-dedup pass is re-enabled (run_command patch) and matmuls
sharing a stationary operand are emitted adjacently so the reload elides.
"""

import numpy as np

import concourse.bass as bass
import concourse.tile as tile
from concourse import mybir
from concourse.bass_utils import run_bass_kernel_spmd
from concourse import bass_utils as _bu

if not getattr(_bu, "_ldw_opt_patch", False):
    _orig_run_command = _bu.run_command

    def _patched_run_command(argv, **kw):
        argv = ["--enable-ldw-opt=true" if a == "--enable-ldw-opt=false" else a
                for a in argv]
        return _orig_run_command(argv, **kw)

    _bu.run_command = _patched_run_command
    _bu._ldw_opt_patch = True

F32R = mybir.dt.float32r
F32 = mybir.dt.float32

N_CORES = 8
B, T, D, H = 2, 4096, 512, 8
DK = D // H          # 64
TT = T // 128        # 32 kv tiles
KC = D // 128        # 4 contraction chunks
QS = 1024            # q super-block (exp granularity)
NC2 = QS // 512      # 512-wide q chunks per super
NQS = T // QS        # supers per head
VW = 132             # vaug cols per kv tile: [v_h0(64) one one v_h1(64) one one]

_split_ctr = [0]


def split_excess_waits(nc, limit=1):
    """walrus codegen in this toolchain accepts at most `limit` sync waits
    per instruction; move the excess onto nofuse NoOps inserted right before
    on the same engine (engines execute in order, semantics unchanged)."""
    n_split = 0
    for fn in nc.m.functions:
        blocks = fn.blocks if isinstance(fn.blocks, list) else list(fn.blocks.values())
        for blk in blocks:
            out = []
            for inst in blk.instructions:
                si = inst.sync_info
                if si is not None and len(si.on_wait) > limit:
                    waits = list(si.on_wait)
                    excess, keep = waits[:-limit], waits[-limit:]
                    for w in excess:
                        _split_ctr[0] += 1
                        out.append(mybir.InstNoOp(
                            name=f"I-wsplit-{_split_ctr[0]}",
                            opcode="NoOp",
                            engine=inst.engine,
                            sync_info=mybir.SyncInfo(on_wait=[w], on_update=[]),
                            bass_nofuse=True,
                        ))
                        n_split += 1
                    inst.sync_info = mybir.SyncInfo(
                        on_wait=keep, on_update=list(si.on_update))
                out.append(inst)
            blk.instructions[:] = out
    return n_split


def _bcast_ap(src_row, nparts):
    """Stride-0 partition broadcast view of a [1, N] AP (DRAM source only)."""
    return bass.AP(
        tensor=src_row.tensor,
        offset=src_row.offset,
        ap=[[0, nparts]] + [list(d) for d in src_row.ap[1:]],
    )


def build_kernel():
    nc = bass.Bass()
    xbT = nc.dram_tensor("xbT", [D, T], F32R, kind="ExternalInput")
    wqT = nc.dram_tensor("wqT", [D, 128], F32R, kind="ExternalInput")
    wkT = nc.dram_tensor("wkT", [D, 128], F32R, kind="ExternalInput")
    wvT = nc.dram_tensor("wvT", [D, 128], F32R, kind="ExternalInput")
    woT = nc.dram_tensor("woT", [128, D], F32R, kind="ExternalInput")
    bq = nc.dram_tensor("bq", [128, 1], F32, kind="ExternalInput")
    bk = nc.dram_tensor("bk", [128, 1], F32, kind="ExternalInput")
    part = nc.dram_tensor("part", [T, D], F32, kind="ExternalOutput")

    with tile.TileContext(nc) as tc:
        with tc.tile_pool(name="persist", bufs=1) as persist:
            # ---- persistent SBUF ----
            woTs = persist.tile([128, D], F32R)
            nc.sync.dma_start(out=woTs, in_=woT[:, :])
            bq_t = persist.tile([128, 1], F32)
            nc.sync.dma_start(out=bq_t, in_=bq[:, :])
            bk_t = persist.tile([128, 1], F32)
            nc.sync.dma_start(out=bk_t, in_=bk[:, :])
            wqt = persist.tile([128, KC, 128], F32R)
            nc.sync.dma_start(out=wqt, in_=wqT.rearrange("(c p) m -> p c m", p=128))
            wkt = persist.tile([128, KC, 128], F32R)
            nc.sync.dma_start(out=wkt, in_=wkT.rearrange("(c p) m -> p c m", p=128))
            wvt = persist.tile([128, KC, 128], F32R)
            nc.sync.dma_start(out=wvt, in_=wvT.rearrange("(c p) m -> p c m", p=128))
            ones2 = persist.tile([128, 2], F32)
            nc.vector.memset(ones2, 1.0)

            qT2 = persist.tile([128, T], F32R)   # heads stacked [h0|h1]
            # k stationaries zero-padded to 128 contraction rows per head:
            # HAM's activity monitor ignores 64-row matmuls (PE stays clocked
            # at 1.2 GHz); 128-row matmuls keep it at 2.4 GHz. kT2z[0] holds
            # [k_h0; 0], kT2z[1] holds [0; k_h1]; the padded rows multiply
            # the other head's q values by zero.
            kT2z = [persist.tile([128, T], F32R, name=f"kT2z{h}")
                    for h in range(2)]
            vaug = persist.tile([128, TT * VW], F32R)
            ctxT2 = persist.tile([128, T], F32R)
            nc.vector.memset(kT2z[0].bitcast(F32)[64:128, :], 0.0)
            nc.vector.memset(kT2z[1].bitcast(F32)[0:64, :], 0.0)

            # ---- stage A/B: load xT (chunked, pipelined) + projections ----
            with tc.tile_pool(name="xT", bufs=1) as xTp:
                xTall = xTp.tile([128, KC * T], F32R)  # chunk c at cols [c*T,...)
                with tc.tile_pool(name="psB", bufs=2, space="PSUM") as psB, \
                     tc.tile_pool(name="psV", bufs=2, space="PSUM") as psV:
                    for n in range(T // 512):
                        sl = slice(512 * n, 512 * (n + 1))
                        for c in range(KC):
                            nc.sync.dma_start(
                                out=xTall[:, c * T + 512 * n: c * T + 512 * (n + 1)],
                                in_=xbT[128 * c: 128 * (c + 1), sl])
                        ps_q = psB.tile([128, 512], F32, tag="psq")
                        for c in range(KC):
                            nc.tensor.matmul(
                                ps_q, wqt[:, c, :],
                                xTall[:, c * T + 512 * n: c * T + 512 * (n + 1)],
                                start=(c == 0), stop=(c == KC - 1))
                        nc.vector.tensor_scalar_add(
                            out=qT2[:, sl], in0=ps_q, scalar1=bq_t)
                        ps_k = psB.tile([128, 512], F32, tag="psk")
                        for c in range(KC):
                            nc.tensor.matmul(
                                ps_k, wkt[:, c, :],
                                xTall[:, c * T + 512 * n: c * T + 512 * (n + 1)],
                                start=(c == 0), stop=(c == KC - 1))
                        nc.vector.tensor_scalar_add(
                            out=kT2z[0][0:64, sl], in0=ps_k[0:64, :],
                            scalar1=bk_t[0:64, :])
                        nc.vector.tensor_scalar_add(
                            out=kT2z[1][64:128, sl], in0=ps_k[64:128, :],
                            scalar1=bk_t[64:128, :])
                    for i in range(TT):
                        ps_v = psV.tile([128, 128], F32, tag="psv")
                        for c in range(KC):
                            nc.tensor.matmul(
                                ps_v,
                                xTall[:, c * T + 128 * i: c * T + 128 * (i + 1)],
                                wvt[:, c, :],
                                start=(c == 0), stop=(c == KC - 1))
                        nc.vector.tensor_copy(
                            out=vaug[:, VW * i: VW * i + 64], in_=ps_v[:, 0:64])
                        nc.vector.tensor_copy(
                            out=vaug[:, VW * i + 66: VW * i + 130],
                            in_=ps_v[:, 64:128])
                        nc.vector.tensor_copy(
                            out=vaug[:, VW * i + 64: VW * i + 66], in_=ones2)
                        nc.vector.tensor_copy(
                            out=vaug[:, VW * i + 130: VW * i + 132], in_=ones2)

            # ---- stage C: flash attention per head ----
            with tc.tile_pool(name="stp", bufs=2, space="PSUM") as stp, \
                 tc.tile_pool(name="ctxp", bufs=2, space="PSUM") as ctxp, \
                 tc.tile_pool(name="ptp", bufs=8) as ptp, \
                 tc.tile_pool(name="drp", bufs=4, space="DRAM") as drp, \
                 tc.tile_pool(name="sC", bufs=4) as sC:
                for h in range(2):
                    hq = 64 * h
                    for qi in range(NQS):
                        qoff = QS * qi
                        ps_cts = []
                        for c2 in range(NC2):
                            ps_cts.append(ctxp.tile(
                                [66, 512], F32, tag=f"ctxT{c2}",
                                name=f"psct_{h}_{qi}_{c2}"))
                        for kb in range(TT):
                            st = stp.tile([128, QS], F32, tag="st")
                            for c2 in range(NC2):
                                nc.tensor.matmul(
                                    st[:, 512 * c2: 512 * (c2 + 1)],
                                    kT2z[h][:, 128 * kb: 128 * (kb + 1)],
                                    qT2[:,
                                        qoff + 512 * c2: qoff + 512 * (c2 + 1)],
                                    start=True, stop=True)
                            pt = ptp.tile([128, QS], F32R, tag="pt")
                            nc.scalar.activation(
                                out=pt, in_=st,
                                func=mybir.ActivationFunctionType.Exp)
                            for c2 in range(NC2):
                                nc.tensor.matmul(
                                    ps_cts[c2],
                                    vaug[:, VW * kb + 66 * h: VW * kb + 66 * h + 66],
                                    pt[:, 512 * c2: 512 * (c2 + 1)],
                                    start=(kb == 0), stop=(kb == TT - 1))
                        for c2 in range(NC2):
                            rec2 = sC.tile([2, 512], F32, tag="rec2")
                            nc.vector.reciprocal(rec2, ps_cts[c2][64:66, :])
                            drec = drp.tile([2, 512], F32, tag="drec")
                            nc.sync.dma_start(out=drec, in_=rec2)
                            rbc = sC.tile([64, 512], F32, tag="rbc")
                            nc.gpsimd.dma_start(
                                out=rbc, in_=_bcast_ap(drec[0:1, :], 64))
                            nc.vector.tensor_mul(
                                out=ctxT2[hq:hq + 64,
                                          qoff + 512 * c2: qoff + 512 * (c2 + 1)],
                                in0=ps_cts[c2][0:64, :], in1=rbc)

            # ---- stage D: output projection ----
            with tc.tile_pool(name="psD", bufs=3, space="PSUM") as psD, \
                 tc.tile_pool(name="sD", bufs=4) as sD:
                for g in range(8):
                    ost = sD.tile([128, 4, 512], F32, tag="ost")
                    for a in range(4):
                        i = 4 * g + a
                        ps_o = psD.tile([128, 512], F32, tag="pso")
                        nc.tensor.matmul(
                            ps_o, ctxT2[:, 128 * i: 128 * (i + 1)], woTs,
                            start=True, stop=True)
                        nc.vector.tensor_copy(out=ost[:, a, :], in_=ps_o)
                    nc.sync.dma_start(
                        out=part[512 * g: 512 * (g + 1), :].rearrange(
                            "(a p) d -> p a d", p=128),
                        in_=ost)

    split_excess_waits(nc)
    return nc


_NC_CACHE = None


def _get_nc():
    global _NC_CACHE
    if _NC_CACHE is None:
        _NC_CACHE = build_kernel()
    return _NC_CACHE


def make_in_maps(x, Wq, bq, Wk, bk, Wv, bv, Wo, bo):
    scale = 1.0 / np.sqrt(DK)
    in_maps = []
    for core in range(N_CORES):
        b, hp = divmod(core, 4)
        R = slice(128 * hp, 128 * hp + 128)
        in_maps.append({
            "xbT": np.ascontiguousarray(x[b].T, dtype=np.float32),
            "wqT": np.ascontiguousarray((Wq[R] * scale).T, dtype=np.float32),
            "wkT": np.ascontiguousarray(Wk[R].T, dtype=np.float32),
            "wvT": np.ascontiguousarray(Wv[R].T, dtype=np.float32),
            "woT": np.ascontiguousarray(Wo[:, R].T, dtype=np.float32),
            "bq": np.ascontiguousarray(
                (bq[R] * scale).reshape(128, 1), dtype=np.float32),
            "bk": np.ascontiguousarray(bk[R].reshape(128, 1), dtype=np.float32),
        })
    return in_maps


def kernel(x, Wq, bq, Wk, bk, Wv, bv, Wo, bo):
    x = np.asarray(x, dtype=np.float32)
    Wq, Wk, Wv, Wo = (np.asarray(a, dtype=np.float32) for a in (Wq, Wk, Wv, Wo))
    bq, bk, bv, bo = (np.asarray(a, dtype=np.float32) for a in (bq, bk, bv, bo))

    nc = _get_nc()
    in_maps = make_in_maps(x, Wq, bq, Wk, bk, Wv, bv, Wo, bo)
    res = run_bass_kernel_spmd(nc, in_maps, list(range(N_CORES)))
    parts = [res.results[c]["part"] for c in range(N_CORES)]

    bcorr = (bv @ Wo.T + bo).astype(np.float32)  # exact bv/bo contribution
    out = np.empty((B, T, D), dtype=np.float32)
    for b in range(B):
        acc = parts[4 * b].astype(np.float64)
        for c in range(4 * b + 1, 4 * b + 4):
            acc += parts[c]
        out[b] = (acc + bcorr).astype(np.float32)
    return out

